# revision 9
# baseline (speedup 1.0000x reference)
"""BiDirectionalAttention fused kernel for 8 Trainium2 NeuronCores.

Shapes (hardcoded): H [1, 16384, 1024], U [1, 1024, 1024], d=1024.
Sharding: context rows (c_len=16384) split 8 ways -> 2048 rows per core.

Per-core device program (scores computed transposed, [q, c], so the
second matmul needs no on-chip transposes):
  sT[q,c] = (w_qc*U)  @ H_loc^T  + uq[q] + hc[c] + bsum   (rank-2 bias fold
             as a K=2 matmul into the same PSUM accumulation group)
  P = exp(sT)          (no max subtraction: |s| <~ 15, safe in fp32)
  r[c] = sum_q P       (ones-column matmul alongside the U_tog matmul)
  U_tog = P^T @ U / r
  g[c] = max_q P = exp(b[c]); per-core stats gmax, S=sum(g/gmax),
  v = (g/gmax) @ H_loc
Host combines the 8 cores' (gmax, S, v) in linear space (equivalent to
the c2q softmax all-reduce) and broadcasts H_row.

Matmuls run as float32r (full-rate PE path for 4-byte floats).
"""

import sys

if "/opt/trn_rl_repo" not in sys.path:
    sys.path.insert(0, "/opt/trn_rl_repo")

from contextlib import ExitStack

import numpy as np

N_CORES = 8
C, Q, D = 16384, 1024, 1024
CL = C // N_CORES      # 2048 context rows per core
NQT = Q // 128         # 8 q tiles
NDC = D // 128         # 8 d chunks
CCW = 512              # c chunk width
NCC = CL // CCW        # 4 c chunks
NCT = CL // 128        # 16 c tiles

_CACHE = {}


def _build_program():
    import concourse.bass as bass  # noqa: F401
    import concourse.mybir as mybir
    import concourse.tile as tile
    from concourse import bacc
    from concourse.masks import make_identity

    dt = mybir.dt
    AF = mybir.ActivationFunctionType
    AX = mybir.AxisListType
    f32 = dt.float32
    f32r = dt.float32r

    nc = bacc.Bacc("TRN2", target_bir_lowering=False, debug=False,
                   num_devices=N_CORES)

    hT = nc.dram_tensor("hT", [D, CL], f32r, kind="ExternalInput").ap()
    h = nc.dram_tensor("h", [CL, D], f32r, kind="ExternalInput").ap()
    uT = nc.dram_tensor("uT", [D, Q], f32r, kind="ExternalInput").ap()
    u = nc.dram_tensor("u", [Q, D], f32r, kind="ExternalInput").ap()
    wqc = nc.dram_tensor("wqc", [128, NDC], f32, kind="ExternalInput").ap()
    wq = nc.dram_tensor("wq", [128, NDC], f32r, kind="ExternalInput").ap()
    wc = nc.dram_tensor("wc", [128, NDC], f32r, kind="ExternalInput").ap()
    bs = nc.dram_tensor("bs", [1, 1], f32, kind="ExternalInput").ap()
    ones1 = nc.dram_tensor("ones1", [1, Q], f32r, kind="ExternalInput").ap()
    u_out = nc.dram_tensor("u_out", [CL, D], f32, kind="ExternalOutput").ap()
    v_out = nc.dram_tensor("v_out", [1, D], f32, kind="ExternalOutput").ap()
    g_out = nc.dram_tensor("g_out", [1, 1], f32, kind="ExternalOutput").ap()
    s_out = nc.dram_tensor("s_out", [1, 1], f32, kind="ExternalOutput").ap()

    with tile.TileContext(nc) as tc, ExitStack() as ctx:
        const = ctx.enter_context(tc.tile_pool(name="const", bufs=1))
        stream = ctx.enter_context(tc.tile_pool(name="stream", bufs=2))
        ppool = ctx.enter_context(tc.tile_pool(name="ppool", bufs=2))
        outp = ctx.enter_context(tc.tile_pool(name="outp", bufs=3))
        small = ctx.enter_context(tc.tile_pool(name="small", bufs=4))
        gmp = ctx.enter_context(tc.tile_pool(name="gmp", bufs=2))
        ps_s = ctx.enter_context(tc.tile_pool(name="ps_s", bufs=2, space="PSUM"))
        ps_w = ctx.enter_context(tc.tile_pool(name="ps_w", bufs=1, space="PSUM"))
        ps_1 = ctx.enter_context(tc.tile_pool(name="ps_1", bufs=1, space="PSUM"))
        ps_tr = ctx.enter_context(tc.tile_pool(name="ps_tr", bufs=1, space="PSUM"))
        ps_row = ctx.enter_context(tc.tile_pool(name="ps_row", bufs=1, space="PSUM"))

        # ---- constants / shared operands ----
        uT_sb = const.tile([128, NDC, Q], f32r)
        nc.sync.dma_start(out=uT_sb[:], in_=uT.rearrange("(t p) q -> p t q", p=128))
        u_sb = const.tile([128, NQT, D], f32r)
        nc.sync.dma_start(out=u_sb[:], in_=u.rearrange("(t p) d -> p t d", p=128))
        wqc_sb = const.tile([128, NDC], f32)
        nc.sync.dma_start(out=wqc_sb[:], in_=wqc)
        wq_sb = const.tile([128, NDC], f32r)
        nc.sync.dma_start(out=wq_sb[:], in_=wq)
        wc_sb = const.tile([128, NDC], f32r)
        nc.sync.dma_start(out=wc_sb[:], in_=wc)
        bs_sb = const.tile([1, 1], f32)
        nc.sync.dma_start(out=bs_sb[:], in_=bs)
        ident = const.tile([128, 128], f32)
        make_identity(nc, ident[:])
        ones_col = const.tile([128, 1], f32)
        nc.vector.memset(ones_col[:], 1.0)
        ones_row = const.tile([1, 128], f32)
        nc.vector.memset(ones_row[:], 1.0)
        # uqb: partition 0 = ones, partition 1 = uq (compute engines cannot
        # write at partition base 1, so uq lands there via an SBUF DMA)
        uqb = const.tile([2, NQT, 128], f32r)
        nc.sync.dma_start(out=uqb[0:1, :, :],
                          in_=ones1.rearrange("p (t x) -> p t x", t=NQT))
        g_pp = const.tile([128, NCT], f32)
        e_pp = const.tile([128, NCT], f32r)

        # uq[q] = U @ w_q, computed from U^T before the in-place w_qc scale
        ps_uq = ps_row.tile([1, Q], f32, tag="row")
        for half in range(2):
            sl = slice(half * 512, (half + 1) * 512)
            for td in range(NDC):
                nc.tensor.matmul(ps_uq[0:1, sl], wq_sb[:, td:td + 1],
                                 uT_sb[:, td, sl],
                                 start=(td == 0), stop=(td == NDC - 1))
        uq_row = const.tile([1, Q], f32r)
        nc.scalar.activation(out=uq_row[:], in_=ps_uq[0:1, :], func=AF.Copy)
        nc.sync.dma_start(out=uqb[1:2, :, :],
                          in_=uq_row[:].rearrange("p (t x) -> p t x", t=NQT))

        # scale U^T rows by w_qc in place -> (w_qc * U)^T
        for td in range(NDC):
            nc.vector.tensor_scalar_mul(uT_sb[:, td, :], uT_sb[:, td, :],
                                        wqc_sb[:, td:td + 1])

        hT_re = hT.rearrange("(t p) c -> p t c", p=128)

        for cc in range(NCC):
            csl = slice(cc * CCW, (cc + 1) * CCW)
            hT_sb = stream.tile([128, NDC, CCW], f32r, tag="hT")
            nc.sync.dma_start(out=hT_sb[:], in_=hT_re[:, :, csl])

            # hc row for this chunk, + bsum, packed under a ones row
            ps_hc = ps_row.tile([1, CCW], f32, tag="row")
            for td in range(NDC):
                nc.tensor.matmul(ps_hc[0:1, :], wc_sb[:, td:td + 1],
                                 hT_sb[:, td, :],
                                 start=(td == 0), stop=(td == NDC - 1))
            # hcb: partition 0 = hc + bsum, partition 1 = ones
            hcb = small.tile([2, CCW], f32r, tag="hcb")
            nc.sync.dma_start(out=hcb[1:2, :], in_=ones1[0:1, 0:CCW])
            nc.scalar.activation(out=hcb[0:1, :], in_=ps_hc[0:1, :],
                                 func=AF.Identity, bias=bs_sb[0:1, :])

            # scores (transposed) + exp
            pt_sb = ppool.tile([128, NQT, CCW], f32r, tag="pt")
            for qt in range(NQT):
                ps = ps_s.tile([128, CCW], f32, tag="s")
                qsl = slice(qt * 128, (qt + 1) * 128)
                for td in range(NDC):
                    nc.tensor.matmul(ps[:], uT_sb[:, td, qsl],
                                     hT_sb[:, td, :],
                                     start=(td == 0), stop=False)
                nc.tensor.matmul(ps[:], uqb[:, qt, :], hcb[:],
                                 start=False, stop=True)
                nc.scalar.activation(out=pt_sb[:, qt, :], in_=ps[:], func=AF.Exp)

            # g[c] = max_q P : max over the 8 q-planes, then PE-transpose
            # each 128-block and reduce along the free axis
            gm = gmp.tile([128, CCW], f32, tag="gm")
            nc.vector.tensor_max(gm[:], pt_sb[:, 0, :], pt_sb[:, 1, :])
            for qt in range(2, NQT):
                nc.vector.tensor_max(gm[:], gm[:], pt_sb[:, qt, :])
            for j in range(4):
                ci = cc * 4 + j
                tr = ps_tr.tile([128, 128], f32, tag="tr")
                nc.tensor.transpose(tr[:], gm[:, j * 128:(j + 1) * 128], ident[:])
                nc.vector.reduce_max(out=g_pp[:, ci:ci + 1], in_=tr[:], axis=AX.X)

            # U_tog tiles: P^T @ U with a ones-column for the row sums
            for j in range(4):
                ci = cc * 4 + j
                ps_ut = ps_w.tile([128, D], f32, tag="ut")
                ps_r = ps_1.tile([128, 1], f32, tag="r1")
                for qt in range(NQT):
                    lhsT = pt_sb[:, qt, j * 128:(j + 1) * 128]
                    st, sp = qt == 0, qt == NQT - 1
                    nc.tensor.matmul(ps_ut[:, 0:512], lhsT,
                                     u_sb[:, qt, 0:512], start=st, stop=sp)
                    nc.tensor.matmul(ps_ut[:, 512:1024], lhsT,
                                     u_sb[:, qt, 512:1024], start=st, stop=sp)
                    nc.tensor.matmul(ps_r[:], lhsT.bitcast(f32), ones_col[:],
                                     start=st, stop=sp)
                rinv = small.tile([128, 1], f32, tag="rinv")
                nc.vector.reciprocal(rinv[:], ps_r[:])
                ut_sb = outp.tile([128, D], f32, tag="ut_sb")
                nc.scalar.activation(out=ut_sb[:], in_=ps_ut[:], func=AF.Copy,
                                     scale=rinv[:])
                nc.sync.dma_start(out=u_out[ci * 128:(ci + 1) * 128, :],
                                  in_=ut_sb[:])

        # ---- c2q local stats ----
        gmax_c = small.tile([128, 1], f32, tag="gmax_c")
        nc.vector.reduce_max(out=gmax_c[:], in_=g_pp[:], axis=AX.X)
        tr2 = ps_tr.tile([1, 128], f32, tag="tr")
        nc.tensor.transpose(tr2[:], gmax_c[:], ident[:])
        gmax_s = small.tile([1, 1], f32, tag="gmax_s")
        nc.vector.reduce_max(out=gmax_s[:], in_=tr2[0:1, :], axis=AX.X)
        ps_gb = ps_1.tile([128, 1], f32, tag="r1")
        nc.tensor.matmul(ps_gb[:], ones_row[:], gmax_s[:],
                         start=True, stop=True)
        rg = small.tile([128, 1], f32, tag="rinv")
        nc.vector.reciprocal(rg[:], ps_gb[:])
        nc.vector.tensor_scalar_mul(e_pp[:], g_pp[:], rg[:])
        es = small.tile([128, 1], f32, tag="es")
        nc.vector.reduce_sum(out=es[:], in_=e_pp[:], axis=AX.X)
        ps_S = ps_row.tile([1, 1], f32, tag="row")
        nc.tensor.matmul(ps_S[:], es[:], ones_col[:], start=True, stop=True)
        S_sb = small.tile([1, 1], f32, tag="S_sb")
        nc.scalar.activation(out=S_sb[:], in_=ps_S[:], func=AF.Copy)

        # v = e @ H_loc
        ps_v = ps_row.tile([1, D], f32, tag="row")
        for ci in range(NCT):
            h_sb = stream.tile([128, D], f32r, tag="h")
            nc.sync.dma_start(out=h_sb[:], in_=h[ci * 128:(ci + 1) * 128, :])
            st, sp = ci == 0, ci == NCT - 1
            nc.tensor.matmul(ps_v[0:1, 0:512], e_pp[:, ci:ci + 1],
                             h_sb[:, 0:512], start=st, stop=sp)
            nc.tensor.matmul(ps_v[0:1, 512:1024], e_pp[:, ci:ci + 1],
                             h_sb[:, 512:1024], start=st, stop=sp)
        v_sb = small.tile([1, D], f32, tag="v_sb")
        nc.scalar.activation(out=v_sb[:], in_=ps_v[0:1, :], func=AF.Copy)

        nc.sync.dma_start(out=v_out[:], in_=v_sb[:])
        nc.sync.dma_start(out=g_out[:], in_=gmax_s[:])
        nc.sync.dma_start(out=s_out[:], in_=S_sb[:])

    nc.compile()
    return nc


def _get_program():
    if "nc" not in _CACHE:
        _CACHE["nc"] = _build_program()
    return _CACHE["nc"]


def kernel(H, U, w_q, b_q, w_c, b_c, w_qc, b_qc):
    from concourse.bass_utils import run_bass_kernel_spmd

    H = np.asarray(H, dtype=np.float32)
    U = np.asarray(U, dtype=np.float32)
    w_q = np.asarray(w_q, dtype=np.float32)
    w_c = np.asarray(w_c, dtype=np.float32)
    w_qc = np.asarray(w_qc, dtype=np.float32)
    bsum = np.array([[float(np.asarray(b_q).reshape(-1)[0])
                      + float(np.asarray(b_c).reshape(-1)[0])
                      + float(np.asarray(b_qc).reshape(-1)[0])]], dtype=np.float32)

    H0, U0 = H[0], U[0]
    uT_np = np.ascontiguousarray(U0.T)
    wqc_np = np.ascontiguousarray(w_qc.reshape(NDC, 128).T)
    wq_np = np.ascontiguousarray(w_q.reshape(NDC, 128).T)
    wc_np = np.ascontiguousarray(w_c.reshape(NDC, 128).T)

    in_maps = []
    for k in range(N_CORES):
        sh = H0[k * CL:(k + 1) * CL]
        in_maps.append({
            "hT": np.ascontiguousarray(sh.T),
            "h": np.ascontiguousarray(sh),
            "uT": uT_np,
            "u": np.ascontiguousarray(U0),
            "wqc": wqc_np,
            "wq": wq_np,
            "wc": wc_np,
            "bs": bsum,
            "ones1": np.ones((1, Q), dtype=np.float32),
        })

    nc = _get_program()
    res = run_bass_kernel_spmd(nc, in_maps, list(range(N_CORES)))

    u_parts = [res.results[k]["u_out"] for k in range(N_CORES)]
    g = np.array([res.results[k]["g_out"][0, 0] for k in range(N_CORES)],
                 dtype=np.float64)
    S = np.array([res.results[k]["s_out"][0, 0] for k in range(N_CORES)],
                 dtype=np.float64)
    v = np.stack([res.results[k]["v_out"][0] for k in range(N_CORES)]).astype(
        np.float64)

    w = g / g.max()
    V = (w[:, None] * v).sum(0)
    H_row = (V / (w * S).sum()).astype(np.float32)

    U_toggler = np.concatenate(u_parts, axis=0)[None]
    H_toggler = np.broadcast_to(H_row[None, None, :], (1, C, D)).copy()
    return (U_toggler, H_toggler)
